# revision 9
# baseline (speedup 1.0000x reference)
"""Trainium2 Bass kernel for the DARTS-style mixed-op network (moe_routing).

Strategy: data-parallel over batch across 8 NeuronCores (weights replicated).
Per core: batch 512, feature-major activation layout [128, 8*512] resident in
SBUF; weights streamed from HBM as bf16 in per-(edge, o_tile) chunks laid out
in exact consumption order; matmuls in bf16 (full PE rate) with fp32 PSUM
accumulation; per-op epilogue on ACT (relu/tanh/sigmoid with per-partition
bias) and DVE (weighted sum). Genotype weights are positive, so identity /
relu / leaky_relu op weights are folded into the weights+biases on the host;
leaky_relu(z) = 0.8*relu(z) + 0.2*z; tanh/sigmoid weights are baked as DVE
immediates.
"""
import sys

if '/opt/trn_rl_repo' not in sys.path:
    sys.path.insert(0, '/opt/trn_rl_repo')

import numpy as np
import ml_dtypes

NUM_NODES = 4
D = 1024
NUM_OPS = 5
NUM_LAYERS = 18
BATCH = 4096
N_CORES = 8
B_C = BATCH // N_CORES          # 512 batch per core
N_OT = D // 128                 # 8 output-feature tiles per layer
N_BT = B_C // 512               # 1 batch tile of 512 per core

# matmul dtype for weights/activations
MM_DTYPE_NP = ml_dtypes.bfloat16


def _edge_list():
    """Mirror the reference loop: per node, edges over current states."""
    edges = []
    si = bi = 0
    nstates = 3
    srcs = ['s0', 's1', 'cat', 'n0', 'n1', 'n2']
    for i in range(NUM_NODES):
        for j in range(nstates):
            l = len(edges)
            if j == 2:
                wkey = ('Wb', bi); bi += 1
            else:
                wkey = ('Ws', si); si += 1
            edges.append(dict(l=l, node=i, src=srcs[j], wkey=wkey,
                              last_of_node=(j == nstates - 1)))
        nstates += 1
    return edges


EDGES = _edge_list()


def _prep_host(s_0, s_1, genotype, Ws, Wb, bs):
    """Build device-side input arrays: weight chunk buffers, biases, transposed
    activations per core."""
    g = np.asarray(genotype, np.float64)

    # Per-edge effective weights/biases. k in {0 (identity), 1 (relu), 4 (leaky)}
    # are prescaled by genotype (positive weights commute with relu/leaky).
    chunksS = []   # normal edges: [128, 5*8*128] per (edge, o_t)
    chunksB = []   # cat edges:    [128, 5*16*128] per (edge, o_t)
    bias_cols = np.zeros((128, NUM_LAYERS * NUM_OPS * N_OT), np.float32)
    w23 = []       # per edge: (w2, w3) immediates

    for e in EDGES:
        l = e['l']
        W = np.asarray(Ws[e['wkey'][1]] if e['wkey'][0] == 'Ws' else Wb[e['wkey'][1]],
                       np.float32)      # [5, Din, 1024]
        b = np.asarray(bs[l], np.float32)  # [5, 1024]
        scale = np.ones(NUM_OPS, np.float32)
        for k in (0, 1, 4):
            scale[k] = g[l, k]
        w23.append((float(g[l, 2]), float(g[l, 3])))

        Din = W.shape[1]
        nKT = Din // 128
        # weight chunks per o_tile: [128, 5*nKT*128], free idx = (k*nKT+i_t)*128+o_loc
        Weff = (W * scale[:, None, None]).astype(MM_DTYPE_NP)  # [5, Din, 1024]
        for o_t in range(N_OT):
            sub = Weff[:, :, o_t * 128:(o_t + 1) * 128]        # [5, Din, 128]
            sub = sub.reshape(NUM_OPS, nKT, 128, 128)          # [k, i_t, p, o_loc]
            sub = sub.transpose(2, 0, 1, 3).reshape(128, NUM_OPS * nKT * 128)
            (chunksS if nKT == 8 else chunksB).append(np.ascontiguousarray(sub))
        beff = (b * scale[:, None]).astype(np.float32)         # [5, 1024]
        for k in range(NUM_OPS):
            cols = beff[k].reshape(N_OT, 128).T                # [128, 8]
            c0 = (l * NUM_OPS + k) * N_OT
            bias_cols[:, c0:c0 + N_OT] = cols

    wbufS = np.ascontiguousarray(np.concatenate(chunksS, axis=0))  # [112*128, 5120]
    wbufB = np.ascontiguousarray(np.concatenate(chunksB, axis=0))  # [32*128, 10240]

    # per-core transposed inputs [128, 8*512]: dev[p, i_t*512+j] = s[c*512+j, i_t*128+p]
    def to_dev(s):
        s = np.asarray(s, np.float32).astype(MM_DTYPE_NP)
        out = []
        for c in range(N_CORES):
            sc = s[c * B_C:(c + 1) * B_C]                      # [512, 1024]
            sc = sc.reshape(B_C, N_OT, 128).transpose(2, 1, 0) # [128, 8, 512]
            out.append(np.ascontiguousarray(sc.reshape(128, N_OT * B_C)))
        return out

    return wbufS, wbufB, bias_cols, w23, to_dev(s_0), to_dev(s_1)


def _build_program(w23, repeat=1):
    """Emit the Bass/Tile program. w23: per-edge (tanh, sigmoid) genotype imms.
    repeat>1 duplicates the whole compute (for slope-based HW timing)."""
    import concourse.bass as bass
    import concourse.mybir as mybir
    import concourse.tile as tile
    from concourse import bacc

    F32 = mybir.dt.float32
    BF16 = mybir.dt.bfloat16
    AF = mybir.ActivationFunctionType
    ALU = mybir.AluOpType

    nS = sum(1 for e in EDGES if e['src'] != 'cat') * N_OT
    nB = sum(1 for e in EDGES if e['src'] == 'cat') * N_OT

    # Bacc (not raw Bass): its compile() lowers multi-wait instructions into
    # event-semaphore sequences walrus can encode (the 64B instruction forms
    # carry a single wait slot).
    nc = bacc.Bacc("TRN2", target_bir_lowering=False, debug=False)
    wbufS_d = nc.dram_tensor("wbufS", [nS * 128, NUM_OPS * 8 * 128], BF16,
                             kind="ExternalInput").ap()
    wbufB_d = nc.dram_tensor("wbufB", [nB * 128, NUM_OPS * 16 * 128], BF16,
                             kind="ExternalInput").ap()
    bias_d = nc.dram_tensor("biasbuf", [128, NUM_LAYERS * NUM_OPS * N_OT], F32,
                            kind="ExternalInput").ap()
    s0_d = nc.dram_tensor("s0t", [128, N_OT * B_C], BF16, kind="ExternalInput").ap()
    s1_d = nc.dram_tensor("s1t", [128, N_OT * B_C], BF16, kind="ExternalInput").ap()
    out_d = nc.dram_tensor("out", [128, N_OT * B_C], F32, kind="ExternalOutput").ap()

    cS = [0]  # chunk counters (mutable closures)
    cB = [0]

    with tile.TileContext(nc) as tc:
        with tc.tile_pool(name="const", bufs=1) as cpool, \
             tc.tile_pool(name="states", bufs=1) as spool, \
             tc.tile_pool(name="accp", bufs=2) as apool, \
             tc.tile_pool(name="wp", bufs=3) as wpool, \
             tc.tile_pool(name="tp", bufs=3) as tpool, \
             tc.tile_pool(name="pp", bufs=8, space="PSUM") as ppool:

            # Stage biases through an ACT-engine copy: ACT consumers of
            # bias_sb then dep on a same-engine instr (no sem wait), keeping
            # every Activation at <=1 sync wait + the one-time table load
            # (the S3D3_AC descriptor has very few wait slots).
            bias_stage = cpool.tile([128, NUM_LAYERS * NUM_OPS * N_OT], F32,
                                    name="bias_stage", tag="bias_stage")
            nc.sync.dma_start(bias_stage[:], bias_d[:])
            bias_sb = cpool.tile([128, NUM_LAYERS * NUM_OPS * N_OT], F32,
                                 name="bias_sb", tag="bias_sb")
            nc.scalar.copy(bias_sb[:], bias_stage[:])

            # Warm the ACT LUT table with a dependency-free Tanh so the
            # walrus-inserted table-load wait lands on an instruction with
            # spare wait slots (AC descriptors hold at most 2 sync waits).
            warm = cpool.tile([128, 1], F32, name="warm", tag="warm")
            nc.vector.memset(warm[:], 0.0)
            nc.scalar.activation(warm[:], warm[:], AF.Tanh)
            s0_sb = spool.tile([128, N_OT * B_C], BF16, name="s0_sb", tag="s0_sb")
            nc.sync.dma_start(s0_sb[:], s0_d[:])
            s1_sb = spool.tile([128, N_OT * B_C], BF16, name="s1_sb", tag="s1_sb")
            nc.sync.dma_start(s1_sb[:], s1_d[:])

            def bias_ap(l, k, o_t):
                c = (l * NUM_OPS + k) * N_OT + o_t
                return bias_sb[:, c:c + 1]

            for _rep in range(repeat):
              states_bf = {'s0': s0_sb, 's1': s1_sb}
              acc = None
              cS[0] = 0
              cB[0] = 0
              for e in EDGES:
                l = e['l']
                node = e['node']
                w2, w3 = w23[l]
                is_cat = e['src'] == 'cat'
                nKT = 16 if is_cat else 8
                first_edge = e['src'] == 's0'
                if first_edge:
                    acc = apool.tile([128, N_OT * B_C], F32, name="acc", tag="acc")

                if is_cat:
                    def rhs_src(i_t):
                        if i_t < 8:
                            return s0_sb[:, i_t * B_C:(i_t + 1) * B_C]
                        return s1_sb[:, (i_t - 8) * B_C:(i_t - 7) * B_C]
                else:
                    sbf = states_bf[e['src']]

                    def rhs_src(i_t, sbf=sbf):
                        return sbf[:, i_t * B_C:(i_t + 1) * B_C]

                for o_t in range(N_OT):
                    # fetch this (edge, o_t) weight chunk (exact consumption order)
                    if is_cat:
                        wch = wpool.tile([128, NUM_OPS * 16 * 128], BF16,
                                         name="wchB", tag="wchB")
                        nc.sync.dma_start(
                            wch[:], wbufB_d[cB[0] * 128:(cB[0] + 1) * 128, :])
                        cB[0] += 1
                    else:
                        wch = wpool.tile([128, NUM_OPS * 8 * 128], BF16,
                                         name="wchS", tag="wchS")
                        nc.sync.dma_start(
                            wch[:], wbufS_d[cS[0] * 128:(cS[0] + 1) * 128, :])
                        cS[0] += 1

                    ps = []
                    for k in range(NUM_OPS):
                        pst = ppool.tile([128, B_C], F32, name="ps", tag="ps")
                        for i_t in range(nKT):
                            lhsT = wch[:, (k * nKT + i_t) * 128:(k * nKT + i_t + 1) * 128]
                            nc.tensor.matmul(pst[:], lhsT=lhsT, rhs=rhs_src(i_t),
                                             start=(i_t == 0), stop=(i_t == nKT - 1))
                        ps.append(pst)

                    # epilogue: out_tile = (h0+b0') + relu(h1+b1') + w2*tanh(h2+b2)
                    #   + w3*sigmoid(h3+b3) + 0.8*relu(h4+b4') + 0.2*(h4+b4')
                    t1 = tpool.tile([128, B_C], F32, name="t1", tag="t1")
                    nc.scalar.activation(t1[:], ps[1][:], AF.Relu, bias=bias_ap(l, 1, o_t))
                    t2 = tpool.tile([128, B_C], F32, name="t2", tag="t2")
                    nc.scalar.activation(t2[:], ps[2][:], AF.Tanh, bias=bias_ap(l, 2, o_t))
                    t3 = tpool.tile([128, B_C], F32, name="t3", tag="t3")
                    nc.scalar.activation(t3[:], ps[3][:], AF.Sigmoid, bias=bias_ap(l, 3, o_t))
                    t4 = tpool.tile([128, B_C], F32, name="t4", tag="t4")
                    nc.scalar.activation(t4[:], ps[4][:], AF.Relu, bias=bias_ap(l, 4, o_t))

                    t5 = tpool.tile([128, B_C], F32, name="t5", tag="t5")
                    # t5 = 0.2*(h4 + b4')
                    nc.vector.tensor_scalar(t5[:], ps[4][:], bias_ap(l, 4, o_t), 0.2,
                                            op0=ALU.add, op1=ALU.mult)
                    # t1 <- (h0 + b0') + t1
                    nc.vector.scalar_tensor_tensor(t1[:], in0=ps[0][:],
                                                   scalar=bias_ap(l, 0, o_t),
                                                   in1=t1[:], op0=ALU.add, op1=ALU.add)
                    # t2 <- w2*t2 + t5
                    nc.vector.scalar_tensor_tensor(t2[:], in0=t2[:], scalar=w2,
                                                   in1=t5[:], op0=ALU.mult, op1=ALU.add)
                    # t4 <- 0.8*t4 + t2
                    nc.vector.scalar_tensor_tensor(t4[:], in0=t4[:], scalar=0.8,
                                                   in1=t2[:], op0=ALU.mult, op1=ALU.add)
                    # t3 <- w3*t3 + t1
                    nc.vector.scalar_tensor_tensor(t3[:], in0=t3[:], scalar=w3,
                                                   in1=t1[:], op0=ALU.mult, op1=ALU.add)

                    acc_sl = acc[:, o_t * B_C:(o_t + 1) * B_C]
                    if first_edge:
                        nc.vector.tensor_tensor(acc_sl, t3[:], t4[:], op=ALU.add)
                    else:
                        nc.vector.tensor_tensor(t3[:], t3[:], t4[:], op=ALU.add)
                        nc.vector.tensor_tensor(acc_sl, acc_sl, t3[:], op=ALU.add)

                    if e['last_of_node'] and node < NUM_NODES - 1:
                        nbf = states_bf.setdefault(
                            f'n{node}',
                            spool.tile([128, N_OT * B_C], BF16,
                                       name=f"nbf{node}", tag=f"nbf{node}"))
                        nc.vector.tensor_copy(
                            nbf[:, o_t * B_C:(o_t + 1) * B_C], acc_sl)

              nc.sync.dma_start(out_d[:], acc[:])

    nc.compile()
    return nc


_MEMO = {}


def _get_program(w23):
    key = tuple(w23)
    if key not in _MEMO:
        _MEMO[key] = _build_program(w23)
    return _MEMO[key]


def kernel(s_0, s_1, genotype, Ws, Wb, bs):
    from concourse.bass_utils import run_bass_kernel_spmd

    wbufS, wbufB, bias_cols, w23, s0_cores, s1_cores = _prep_host(
        s_0, s_1, genotype, Ws, Wb, bs)
    nc = _get_program(w23)

    in_maps = [{
        "wbufS": wbufS,
        "wbufB": wbufB,
        "biasbuf": bias_cols,
        "s0t": s0_cores[c],
        "s1t": s1_cores[c],
    } for c in range(N_CORES)]

    res = run_bass_kernel_spmd(nc, in_maps, core_ids=list(range(N_CORES)))

    out = np.empty((BATCH, D), np.float32)
    for c in range(N_CORES):
        t = res.results[c]["out"]                       # [128, 8*512]
        t = t.reshape(128, N_OT, B_C).transpose(2, 1, 0)  # [512, 8, 128]
        out[c * B_C:(c + 1) * B_C] = t.reshape(B_C, D)
    return out
